# revision 7
# baseline (speedup 1.0000x reference)
"""LSTM caption-decoder kernel for 8 trn2 NeuronCores (Bass/Tile, SPMD).

Sharding: pure data-parallel over batch (16 rows per core, no collectives).
Per core:
  - gather embeddings for its 512 (t,b) rows via indirect DMA
  - mean-pool features -> h0/c0 via matmuls
  - precompute Wx = emb @ W_ih^T + bias  (frees the recurrence from the x-side)
  - 32-step LSTM recurrence: bf16 matmuls (fp32 PSUM accum) + fp32 elementwise;
    h^T produced by PE transposes whose "identity" operand is a per-step
    diagonal 0/1 length-mask (free masking, and finished rows go dark)
  - fc projection in 4 windows of 128 (t,b) rows (M=128) against a resident
    bf16 fc_w^T; fc bias added via a K=1 matmul whose lhsT is the 0/1 mask row,
    so masked rows come out of PSUM as exact zeros.
"""

import sys
import os

if "/opt/trn_rl_repo" not in sys.path:
    sys.path.insert(0, "/opt/trn_rl_repo")

import numpy as np
import ml_dtypes

BF = ml_dtypes.bfloat16

B, T, E, H, V, LF = 128, 32, 512, 512, 10000, 49
NCORES = 8
BS = B // NCORES          # 16 batch rows per core
TB = T * BS               # 512 (t,b) rows per core, row = t*BS + b
GC = 4 * H                # 2048 gate columns
KT = 4                    # k tiles (H/128)
NW = T // 8               # 4 fc windows of 128 rows
VCH = 500                 # vocab chunk (<=512 fp32 psum bank)
NVC = V // VCH            # 20 chunks

# gate column permutation: torch order i,f,g,o -> kernel order i,g,f,o
_PERM = np.concatenate([
    np.arange(0, H), np.arange(2 * H, 3 * H),
    np.arange(H, 2 * H), np.arange(3 * H, 4 * H),
])

_CACHE: dict = {}


def _emit(nc, tc, tile, bass, mybir, d):
    """Emit the per-core program. d: dict of dram APs."""
    from contextlib import ExitStack

    dt = mybir.dt
    f32, bf, i32 = dt.float32, dt.bfloat16, dt.int32
    AF = mybir.ActivationFunctionType

    ctx = ExitStack()
    with ctx:
        psp = ctx.enter_context(tc.tile_pool(name="ps", bufs=1, space="PSUM"))
        cp = ctx.enter_context(tc.tile_pool(name="const", bufs=1))
        wp = ctx.enter_context(tc.tile_pool(name="w", bufs=1))
        sp = ctx.enter_context(tc.tile_pool(name="state", bufs=1))
        wk = ctx.enter_context(tc.tile_pool(name="work", bufs=2))

        # ---- constants / small tensors
        id128 = cp.tile([128, 128], bf)
        nc.sync.dma_start(id128[:], d["id128"])
        id16b = cp.tile([16, 16], bf)
        nc.sync.dma_start(id16b[:], d["id16b"])
        mcol = cp.tile([BS, T], f32)   # mask[b, t] = t < len[b]
        nc.sync.dma_start(mcol[:], d["mcol"])
        id16f = cp.tile([16, 16], f32)
        nc.sync.dma_start(id16f[:], d["id16f"])
        onesr = cp.tile([1, 128], bf)
        nc.sync.dma_start(onesr[:], d["ones"])
        mrow = cp.tile([1, TB], bf)
        nc.sync.dma_start(mrow[:], d["mrow"])
        gbias = cp.tile([1, GC], bf)
        nc.sync.dma_start(gbias[:], d["gbias"])
        initb = cp.tile([1, 2 * H], bf)
        nc.sync.dma_start(initb[:], d["initb"])
        fcb = cp.tile([1, V], bf)
        nc.sync.dma_start(fcb[:], d["fcb"])

        # ---- recurrence weights (input-side weights live in the init pool)
        whh = [wp.tile([128, GC], bf, name=f"whh{k}") for k in range(KT)]
        for k in range(KT):
            nc.sync.dma_start(whh[k][:], d["whh"][128 * k:128 * (k + 1), :])

        # ---- persistent recurrence state
        hall = [sp.tile([128, TB], bf, name=f"hall{k}") for k in range(KT)]
        wx = [sp.tile([128, GC], bf, name=f"wx{m}") for m in range(NW)]
        c_st = [sp.tile([BS, H], f32, name=f"c{i}") for i in range(2)]
        h0T = [sp.tile([128, BS], bf, name=f"h0T{k}") for k in range(KT)]

        # ================= init phase (transient pool) =================
        with tc.tile_pool(name="init", bufs=1) as ip:
            wih = [ip.tile([128, GC], bf, name=f"wih{k}") for k in range(KT)]
            for k in range(KT):
                nc.sync.dma_start(wih[k][:], d["wih"][128 * k:128 * (k + 1), :])
            initw = [ip.tile([128, 2 * H], bf, name=f"initw{k}")
                     for k in range(KT)]
            for k in range(KT):
                nc.sync.dma_start(initw[k][:],
                                  d["initw"][128 * k:128 * (k + 1), :])

            # feature mean-pool: feat [BS, LF*E] bf16, in-place halving adds
            feat = ip.tile([BS, LF * E], bf)
            nc.sync.dma_start(feat[:], d["feat"])
            for sz in (24, 12, 6, 3):
                nc.vector.tensor_add(feat[:, :sz * E], feat[:, :sz * E],
                                     feat[:, sz * E:2 * sz * E])
            nc.vector.tensor_add(feat[:, :E], feat[:, :E], feat[:, E:2 * E])
            nc.vector.tensor_add(feat[:, :E], feat[:, :E], feat[:, 2 * E:3 * E])
            nc.vector.tensor_add(feat[:, :E], feat[:, :E],
                                 feat[:, 48 * E:49 * E])
            mfs = feat

            # mf^T tiles [128, 16]
            mfT = [ip.tile([128, BS], bf, name=f"mfT{k}") for k in range(KT)]
            for k in range(KT):
                tp = psp.tile([128, BS], bf, tag="ht", bufs=2)
                nc.tensor.transpose(tp[:], mfs[:, 128 * k:128 * (k + 1)], id16b[:])
                nc.vector.tensor_copy(mfT[k][:], tp[:])

            # h0 | c0 = mfsum @ (init_w^T/49) + init_b
            hc = psp.tile([BS, 2 * H], f32, tag="gates", bufs=1)
            for ci in range(2):
                sl = slice(H * ci, H * (ci + 1))
                for k in range(KT):
                    nc.tensor.matmul(hc[:, sl], lhsT=mfT[k][:], rhs=initw[k][:, sl],
                                     start=(k == 0), stop=False)
                nc.tensor.matmul(hc[:, sl], lhsT=onesr[:, :BS], rhs=initb[:, sl],
                                 start=False, stop=True)
            h0 = ip.tile([BS, H], f32)
            nc.vector.tensor_copy(h0[:], hc[:, :H])
            nc.vector.tensor_copy(c_st[0][:], hc[:, H:2 * H])
            for k in range(KT):
                tp = psp.tile([128, BS], f32, tag="ht", bufs=2)
                nc.tensor.transpose(tp[:], h0[:, 128 * k:128 * (k + 1)], id16f[:])
                nc.vector.tensor_copy(h0T[k][:], tp[:])

            # embedding gather (indirect DMA), rows in (t,b) order
            idx = [ip.tile([128, 1], i32, name=f"idx{m}") for m in range(NW)]
            embm = [ip.tile([128, E], bf, name=f"embm{m}") for m in range(NW)]
            for m in range(NW):
                nc.sync.dma_start(idx[m][:], d["idx"][128 * m:128 * (m + 1), :])
                nc.gpsimd.indirect_dma_start(
                    out=embm[m][:], out_offset=None,
                    in_=d["embtab"],
                    in_offset=bass.IndirectOffsetOnAxis(ap=idx[m][:, :1], axis=0),
                )
            # emb^T tiles [E-chunk 128, TB]
            embT = [ip.tile([128, TB], bf, name=f"embT{k}") for k in range(KT)]
            for m in range(NW):
                for k in range(KT):
                    tp = psp.tile([128, 128], bf, tag="ht", bufs=2)
                    nc.tensor.transpose(tp[:], embm[m][:, 128 * k:128 * (k + 1)],
                                        id128[:])
                    nc.vector.tensor_copy(embT[k][:, 128 * m:128 * (m + 1)], tp[:])

            # Wx = emb @ W_ih^T + gbias   -> wx[m] [128, GC] bf16
            for m in range(NW):
                for nch in range(4):
                    sl = slice(512 * nch, 512 * (nch + 1))
                    wps = psp.tile([128, 512], f32, tag="fc", bufs=2)
                    for k in range(KT):
                        nc.tensor.matmul(wps[:], lhsT=embT[k][:, 128 * m:128 * (m + 1)],
                                         rhs=wih[k][:, sl], start=(k == 0), stop=False)
                    nc.tensor.matmul(wps[:], lhsT=onesr[:], rhs=gbias[:, sl],
                                     start=False, stop=True)
                    nc.vector.tensor_copy(wx[m][:, sl], wps[:])

        # ---- fc weights (own pool: reuses space released by the init pool)
        fcp = ctx.enter_context(tc.tile_pool(name="fcp", bufs=1))
        fcw = [fcp.tile([128, V], bf, name=f"fcw{k}") for k in range(KT)]
        for k in range(KT):
            nc.sync.dma_start(fcw[k][:], d["fcw"][128 * k:128 * (k + 1), :])

        fout = ctx.enter_context(tc.tile_pool(name="fout", bufs=3))

        def fc_window(w):
            for nv in range(NVC):
                vsl = slice(VCH * nv, VCH * (nv + 1))
                fps = psp.tile([128, VCH], f32, tag="fc", bufs=2)
                for k in range(KT):
                    nc.tensor.matmul(fps[:], lhsT=hall[k][:, 128 * w:128 * (w + 1)],
                                     rhs=fcw[k][:, vsl], start=(k == 0), stop=False)
                nc.tensor.matmul(fps[:], lhsT=mrow[:, 128 * w:128 * (w + 1)],
                                 rhs=fcb[:, vsl], start=False, stop=True)
                osb = fout.tile([128, VCH], f32, tag="fo")
                if nv % 2 == 0:
                    nc.scalar.copy(osb[:], fps[:])
                else:
                    nc.vector.tensor_copy(osb[:], fps[:])
                dst = d["preds"][:, 8 * w:8 * (w + 1), vsl]
                nc.sync.dma_start(dst.rearrange("b t n -> t b n"), osb[:])

        # ================= recurrence =================
        hprev = [h0T[k][:] for k in range(KT)]
        for t in range(T):
            m, j = t // 8, t % 8
            g = psp.tile([BS, GC], f32, tag="gates", bufs=1)
            for ci in range(4):
                sl = slice(512 * ci, 512 * (ci + 1))
                for k in range(KT):
                    nc.tensor.matmul(g[:, sl], lhsT=hprev[k], rhs=whh[k][:, sl],
                                     start=(k == 0), stop=False)
                nc.tensor.matmul(g[:, sl], lhsT=id128[:, 16 * j:16 * (j + 1)],
                                 rhs=wx[m][:, sl], start=False, stop=True)
            # chunk order: 0=i, 1=g, 2=f, 3=o (host permuted the weights)
            sig_i = wk.tile([BS, H], f32, tag="si")
            nc.scalar.activation(sig_i[:], g[:, 0:512], AF.Sigmoid)
            tnh_g = wk.tile([BS, H], f32, tag="tg")
            nc.scalar.activation(tnh_g[:], g[:, 512:1024], AF.Tanh)
            sig_f = wk.tile([BS, H], f32, tag="sf")
            nc.scalar.activation(sig_f[:], g[:, 1024:1536], AF.Sigmoid)
            sig_o = wk.tile([BS, H], f32, tag="so")
            nc.scalar.activation(sig_o[:], g[:, 1536:2048], AF.Sigmoid)

            p1 = wk.tile([BS, H], f32, tag="p1")
            nc.vector.tensor_mul(p1[:], sig_i[:], tnh_g[:])
            p2 = wk.tile([BS, H], f32, tag="p2")
            nc.vector.tensor_mul(p2[:], sig_f[:], c_st[t % 2][:])
            c_new = c_st[(t + 1) % 2]
            nc.vector.tensor_add(c_new[:], p1[:], p2[:])
            tnh_c = wk.tile([BS, H], f32, tag="tc")
            nc.scalar.activation(tnh_c[:], c_new[:], AF.Tanh)
            # h = tanh(c) * mask * sig(o)  (length-mask fused in)
            h = wk.tile([BS, H], f32, tag="h")
            nc.vector.scalar_tensor_tensor(
                h[:], tnh_c[:], mcol[:, t:t + 1], sig_o[:],
                op0=mybir.AluOpType.mult, op1=mybir.AluOpType.mult)

            # h^T lands in hall col block t
            htp = psp.tile([128, 4 * BS], f32, tag="ht", bufs=2)
            for k in range(KT):
                nc.tensor.transpose(htp[:, 16 * k:16 * (k + 1)],
                                    h[:, 128 * k:128 * (k + 1)],
                                    id16f[:])
                nc.vector.tensor_copy(hall[k][:, 16 * t:16 * (t + 1)],
                                      htp[:, 16 * k:16 * (k + 1)])
            hprev = [hall[k][:, 16 * t:16 * (t + 1)] for k in range(KT)]

            if t % 8 == 7:
                fc_window(t // 8)


def _build():
    if "nc" in _CACHE:
        return _CACHE["nc"]
    import concourse.bass as bass
    import concourse.tile as tile
    from concourse import bacc, mybir

    dt = mybir.dt
    nc = bacc.Bacc("TRN2", target_bir_lowering=False, debug=False,
                   num_devices=NCORES)

    def din(name, shape, dty):
        return nc.dram_tensor(name, shape, dty, kind="ExternalInput").ap()

    d = {
        "embtab": din("embtab", [V, E], dt.bfloat16),
        "idx": din("idx", [TB, 1], dt.int32),
        "wih": din("wih", [E, GC], dt.bfloat16),
        "whh": din("whh", [H, GC], dt.bfloat16),
        "gbias": din("gbias", [1, GC], dt.bfloat16),
        "initw": din("initw", [E, 2 * H], dt.bfloat16),
        "initb": din("initb", [1, 2 * H], dt.bfloat16),
        "fcw": din("fcw", [H, V], dt.bfloat16),
        "fcb": din("fcb", [1, V], dt.bfloat16),
        "feat": din("feat", [BS, LF * E], dt.bfloat16),
        "mcol": din("mcol", [BS, T], dt.float32),
        "mrow": din("mrow", [1, TB], dt.bfloat16),
        "id128": din("id128", [128, 128], dt.bfloat16),
        "id16b": din("id16b", [16, 16], dt.bfloat16),
        "id16f": din("id16f", [16, 16], dt.float32),
        "ones": din("ones", [1, 128], dt.bfloat16),
        "preds": nc.dram_tensor("preds", [BS, T, V], dt.float32,
                                kind="ExternalOutput").ap(),
    }

    with tile.TileContext(nc) as tc:
        _emit(nc, tc, tile, bass, mybir, d)
    nc.compile()
    _CACHE["nc"] = nc
    return nc


def _shared_inputs(embedding, W_ih, W_hh, b_ih, b_hh, fc_w, fc_b,
                   init_h_w, init_h_b, init_c_w, init_c_b):
    sh = {}
    sh["embtab"] = np.ascontiguousarray(embedding, dtype=np.float32).astype(BF)
    sh["wih"] = np.ascontiguousarray(W_ih[_PERM].T).astype(BF)
    sh["whh"] = np.ascontiguousarray(W_hh[_PERM].T).astype(BF)
    sh["gbias"] = (b_ih + b_hh)[_PERM][None, :].astype(BF)
    iw = np.concatenate([init_h_w, init_c_w], axis=0)  # [2H, 512]
    sh["initw"] = np.ascontiguousarray(iw.T / np.float32(LF)).astype(BF)
    sh["initb"] = np.concatenate([init_h_b, init_c_b])[None, :].astype(BF)
    sh["fcw"] = np.ascontiguousarray(fc_w.T).astype(BF)
    sh["fcb"] = np.asarray(fc_b)[None, :].astype(BF)
    sh["id128"] = np.eye(128, dtype=np.float32).astype(BF)
    sh["id16b"] = np.eye(16, dtype=np.float32).astype(BF)
    sh["id16f"] = np.eye(16, dtype=np.float32)
    sh["ones"] = np.ones((1, 128), dtype=np.float32).astype(BF)
    return sh


def _core_inputs(sh, features, captions, lengths, ci):
    b0 = BS * ci
    feat = np.ascontiguousarray(features[b0:b0 + BS], dtype=np.float32)
    cap = np.asarray(captions[b0:b0 + BS]).astype(np.int64)
    lens = np.asarray(lengths[b0:b0 + BS]).astype(np.int64)
    m = {}
    m.update(sh)
    m["feat"] = feat.reshape(BS, LF * E).astype(BF)
    # (t,b)-ordered gather indices
    m["idx"] = np.ascontiguousarray(cap.T.reshape(TB, 1)).astype(np.int32)
    # mask[b, t] = t < len[b]
    mask = (np.arange(T)[None, :] < lens[:, None]).astype(np.float32)  # [BS,T]
    m["mcol"] = mask
    m["mrow"] = np.ascontiguousarray(mask.T.reshape(1, TB)).astype(BF)
    return m


def _in_maps(inputs):
    sh = _shared_inputs(
        inputs["embedding"], inputs["W_ih"], inputs["W_hh"], inputs["b_ih"],
        inputs["b_hh"], inputs["fc_w"], inputs["fc_b"], inputs["init_h_w"],
        inputs["init_h_b"], inputs["init_c_w"], inputs["init_c_b"])
    return [
        _core_inputs(sh, inputs["features"], inputs["captions"],
                     inputs["lengths"], ci)
        for ci in range(NCORES)
    ]


def _run(inputs, trace=False):
    from concourse.bass_utils import run_bass_kernel_spmd
    nc = _build()
    res = run_bass_kernel_spmd(nc, _in_maps(inputs), list(range(NCORES)),
                               trace=trace)
    preds = np.concatenate(
        [np.asarray(r["preds"], dtype=np.float32) for r in res.results], axis=0)
    return preds, res


def kernel(**inputs):
    preds, _ = _run(inputs, trace=False)
    return preds


def _timed_runner(nc, in_maps):
    """Build the same shard_map executable run_bass_via_pjrt uses, but keep it
    for repeated timed execution with device-resident inputs."""
    import jax
    import numpy as jnp_np
    from jax.sharding import Mesh, PartitionSpec, NamedSharding
    from jax.experimental.shard_map import shard_map
    from concourse import bass2jax, mybir
    from concourse.bass2jax import _bass_exec_p, partition_id_tensor

    bass2jax.install_neuronx_cc_hook()
    n_cores = len(in_maps)
    partition_name = (nc.partition_id_tensor.name
                      if nc.partition_id_tensor else None)
    in_names, out_names, out_avals, zero_outs = [], [], [], []
    for alloc in nc.m.functions[0].allocations:
        if not isinstance(alloc, mybir.MemoryLocationSet):
            continue
        name = alloc.memorylocations[0].name
        if alloc.kind == "ExternalInput":
            if name != partition_name:
                in_names.append(name)
        elif alloc.kind == "ExternalOutput":
            shape = tuple(alloc.tensor_shape)
            dtype = mybir.dt.np(alloc.dtype)
            out_names.append(name)
            out_avals.append(jax.core.ShapedArray(shape, dtype))
            zero_outs.append(np.zeros(shape, dtype))
    n_params = len(in_names)
    n_outs = len(out_avals)
    param_names = list(in_names)
    in_names = in_names + out_names
    if partition_name is not None:
        in_names.append(partition_name)

    def _body(*args):
        operands = list(args)
        if partition_name is not None:
            operands.append(partition_id_tensor())
        outs = _bass_exec_p.bind(
            *operands, out_avals=tuple(out_avals), in_names=tuple(in_names),
            out_names=tuple(out_names), lowering_input_output_aliases=(),
            sim_require_finite=True, sim_require_nnan=True, nc=nc)
        return tuple(outs)

    devices = jax.devices()[:n_cores]
    mesh = Mesh(np.asarray(devices), ("core",))
    spec = PartitionSpec("core")
    sharded = jax.jit(
        shard_map(_body, mesh=mesh, in_specs=(spec,) * (n_params + n_outs),
                  out_specs=(spec,) * n_outs, check_rep=False),
        donate_argnums=tuple(range(n_params, n_params + n_outs)),
        keep_unused=True)
    sh = NamedSharding(mesh, spec)
    concat_in = [
        jax.device_put(np.concatenate(
            [np.asarray(m[nm]) for m in in_maps], axis=0), sh)
        for nm in param_names
    ]
    zglobal = [np.zeros((n_cores * z.shape[0], *z.shape[1:]), z.dtype)
               for z in zero_outs]

    def run_once():
        zs = [jax.device_put(z, sh) for z in zglobal]
        import time as _t
        jax.block_until_ready(zs)
        t0 = _t.perf_counter()
        out = sharded(*concat_in, *zs)
        jax.block_until_ready(out)
        dt = _t.perf_counter() - t0
        return out, dt

    def unpack(out):
        return [
            {nm: np.asarray(out[i]).reshape(n_cores, *out_avals[i].shape)[c]
             for i, nm in enumerate(out_names)}
            for c in range(n_cores)
        ]

    return run_once, unpack


def bench(inputs, iters=8):
    """Repeated timed HW execution; returns (preds, est_exec_ns).
    No NTFF profiling is available through the axon tunnel in this container,
    so we report min wall-clock of the warm executable minus the min wall of a
    trivial calibration kernel (same path), as the HW exec estimate."""
    nc = _build()
    run_once, unpack = _timed_runner(nc, _in_maps(inputs))
    out = None
    times = []
    for _ in range(iters):
        out, dt = run_once()
        times.append(dt)
    preds = np.concatenate(
        [np.asarray(r["preds"], dtype=np.float32) for r in unpack(out)], axis=0)

    cal = _calibration_times(max(4, iters // 2))
    est = (min(times) - min(cal)) * 1e9
    print(f"[bench] main walls (ms): {[round(t*1e3,2) for t in times]}")
    print(f"[bench] calib walls (ms): {[round(t*1e3,2) for t in cal]}")
    return preds, int(est)


def _calibration_times(iters):
    """Trivial kernel through the identical path to estimate fixed overhead."""
    import concourse.bass as bass
    import concourse.tile as tile
    from concourse import bacc, mybir

    if "cal" not in _CACHE:
        dt = mybir.dt
        nc = bacc.Bacc("TRN2", target_bir_lowering=False, debug=False,
                       num_devices=NCORES)
        x = nc.dram_tensor("x", [128, 128], dt.float32,
                           kind="ExternalInput").ap()
        y = nc.dram_tensor("y", [128, 128], dt.float32,
                           kind="ExternalOutput").ap()
        with tile.TileContext(nc) as tc:
            with tc.tile_pool(name="p", bufs=1) as p:
                t = p.tile([128, 128], dt.float32)
                nc.sync.dma_start(t[:], x)
                nc.sync.dma_start(y, t[:])
        nc.compile()
        _CACHE["cal"] = nc
    ncc = _CACHE["cal"]
    maps = [{"x": np.zeros((128, 128), np.float32)} for _ in range(NCORES)]
    run_once, _ = _timed_runner(ncc, maps)
    return [run_once()[1] for _ in range(iters)]


# revision 19
# speedup vs baseline: 5.9172x; 5.9172x over previous
"""LSTM caption-decoder kernel for 8 trn2 NeuronCores (Bass/Tile, SPMD).

Sharding: pure data-parallel over batch (16 rows per core, no collectives).
Per core:
  - gather embeddings for its 512 (t,b) rows via indirect DMA
  - mean-pool features -> h0/c0 via matmuls
  - precompute Wx = emb @ W_ih^T + bias  (frees the recurrence from the x-side)
  - 32-step LSTM recurrence: bf16 matmuls (fp32 PSUM accum) + fp32 elementwise;
    h^T produced by PE transposes whose "identity" operand is a per-step
    diagonal 0/1 length-mask (free masking, and finished rows go dark)
  - fc projection in 4 windows of 128 (t,b) rows (M=128) against a resident
    bf16 fc_w^T; fc bias added via a K=1 matmul whose lhsT is the 0/1 mask row,
    so masked rows come out of PSUM as exact zeros.
"""

import sys
import os

if "/opt/trn_rl_repo" not in sys.path:
    sys.path.insert(0, "/opt/trn_rl_repo")

import numpy as np
import ml_dtypes

BF = ml_dtypes.bfloat16

B, T, E, H, V, LF = 128, 32, 512, 512, 10000, 49
NCORES = 8
BS = B // NCORES          # 16 batch rows per core
TB = T * BS               # 512 (t,b) rows per core, row = t*BS + b
GC = 4 * H                # 2048 gate columns
KT = 4                    # k tiles (H/128)
NW = T // 8               # 4 fc windows of 128 rows
VCH = 500                 # vocab chunk (<=512 fp32 psum bank)
NVC = V // VCH            # 20 chunks
FCG = 5                   # fc chunks per output DMA group
CBF = 3856 + V            # packed bf16 const blob cols
CF32 = T + 16             # packed f32 const blob cols

# gate column permutation: torch order i,f,g,o -> kernel order i,g,f,o
_PERM = np.concatenate([
    np.arange(0, H), np.arange(2 * H, 3 * H),
    np.arange(H, 2 * H), np.arange(3 * H, 4 * H),
])

_CACHE: dict = {}


def _emit(nc, tc, tile, bass, mybir, d, rep=1):
    for r in range(rep):
        _emit_once(nc, tc, tile, bass, mybir, d, str(r) if rep > 1 else "")


def _emit_once(nc, tc, tile, bass, mybir, d, pfx=""):
    """Emit the per-core program. d: dict of dram APs."""
    from contextlib import ExitStack

    dt = mybir.dt
    f32, bf, i32 = dt.float32, dt.bfloat16, dt.int32
    AF = mybir.ActivationFunctionType

    ctx = ExitStack()
    with ctx:
        psp = ctx.enter_context(tc.tile_pool(name="ps" + pfx, bufs=1,
                                             space="PSUM"))
        cp = ctx.enter_context(tc.tile_pool(name="const" + pfx, bufs=1))
        wp = ctx.enter_context(tc.tile_pool(name="w" + pfx, bufs=1))
        sp = ctx.enter_context(tc.tile_pool(name="state" + pfx, bufs=1))
        wk = ctx.enter_context(tc.tile_pool(name="work" + pfx, bufs=2))

        # ---- constants: two packed blobs (one DMA each)
        bfc = cp.tile([128, CBF], bf)
        nc.sync.dma_start(bfc[:], d["bfconst"])
        f32c = cp.tile([BS, CF32], f32)
        nc.sync.dma_start(f32c[:], d["f32const"])
        id128 = bfc[:, 0:128]
        id16b = bfc[0:16, 128:144]
        onesr = bfc[0:1, 144:272]
        mrow = bfc[0:1, 272:272 + TB]
        gbias = bfc[0:1, 784:784 + GC]
        initb = bfc[0:1, 2832:2832 + 2 * H]
        fcb = bfc[0:1, 3856:3856 + V]
        mcol = f32c[:, 0:T]
        id16f = f32c[:, T:T + 16]

        # ---- recurrence weights: one DMA into a packed tile
        whha = wp.tile([128, KT * GC], bf)
        nc.sync.dma_start(whha[:].rearrange("p (k n) -> p k n", k=KT),
                          d["whh"].rearrange("(k p) n -> p k n", k=KT))
        whh = [whha[:, k * GC:(k + 1) * GC] for k in range(KT)]

        # ---- persistent recurrence state
        hall = [sp.tile([128, TB], bf, name=f"hall{k}") for k in range(KT)]
        wx = [sp.tile([128, GC], bf, name=f"wx{m}") for m in range(NW)]
        c_st = [sp.tile([BS, H], f32, name=f"c{i}") for i in range(2)]
        h0T = [sp.tile([128, BS], bf, name=f"h0T{k}") for k in range(KT)]

        # ================= init phase (transient pool) =================
        with tc.tile_pool(name="init" + pfx, bufs=1) as ip:
            wiha = ip.tile([128, KT * GC], bf)
            nc.sync.dma_start(wiha[:].rearrange("p (k n) -> p k n", k=KT),
                              d["wih"].rearrange("(k p) n -> p k n", k=KT))
            wih = [wiha[:, k * GC:(k + 1) * GC] for k in range(KT)]
            initwa = ip.tile([128, KT * 2 * H], bf)
            nc.sync.dma_start(initwa[:].rearrange("p (k n) -> p k n", k=KT),
                              d["initw"].rearrange("(k p) n -> p k n", k=KT))
            initw = [initwa[:, k * 2 * H:(k + 1) * 2 * H] for k in range(KT)]

            # feature mean-pool: feat [BS, LF*E] bf16, in-place halving adds
            feat = ip.tile([BS, LF * E], bf)
            nc.sync.dma_start(feat[:], d["feat"])
            for sz in (24, 12, 6, 3):
                nc.vector.tensor_add(feat[:, :sz * E], feat[:, :sz * E],
                                     feat[:, sz * E:2 * sz * E])
            nc.vector.tensor_add(feat[:, :E], feat[:, :E], feat[:, E:2 * E])
            nc.vector.tensor_add(feat[:, :E], feat[:, :E], feat[:, 2 * E:3 * E])
            nc.vector.tensor_add(feat[:, :E], feat[:, :E],
                                 feat[:, 48 * E:49 * E])
            mfs = feat

            # mf^T tiles [128, 16]
            mfT = [ip.tile([128, BS], bf, name=f"mfT{k}") for k in range(KT)]
            for k in range(KT):
                tp = psp.tile([128, BS], bf, tag="ht", bufs=2)
                nc.tensor.transpose(tp[:], mfs[:, 128 * k:128 * (k + 1)], id16b[:])
                nc.vector.tensor_copy(mfT[k][:], tp[:])

            # h0 | c0 = mfsum @ (init_w^T/49) + init_b
            hc = psp.tile([BS, 2 * H], f32, tag="gates", bufs=2)
            for ci in range(2):
                sl = slice(H * ci, H * (ci + 1))
                for k in range(KT):
                    nc.tensor.matmul(hc[:, sl], lhsT=mfT[k][:], rhs=initw[k][:, sl],
                                     start=(k == 0), stop=False)
                nc.tensor.matmul(hc[:, sl], lhsT=onesr[:, :BS], rhs=initb[:, sl],
                                 start=False, stop=True)
            h0 = ip.tile([BS, H], f32)
            nc.vector.tensor_copy(h0[:], hc[:, :H])
            nc.vector.tensor_copy(c_st[0][:], hc[:, H:2 * H])
            for k in range(KT):
                tp = psp.tile([128, BS], f32, tag="ht", bufs=2)
                nc.tensor.transpose(tp[:], h0[:, 128 * k:128 * (k + 1)], id16f[:])
                nc.vector.tensor_copy(h0T[k][:], tp[:])

            # embedding gather (indirect DMA), rows in (t,b) order
            idxc = ip.tile([128, NW], i32)
            nc.sync.dma_start(idxc[:], d["idx"])
            embm = [ip.tile([128, E], bf, name=f"embm{m}") for m in range(NW)]
            for m in range(NW):
                nc.gpsimd.indirect_dma_start(
                    out=embm[m][:], out_offset=None,
                    in_=d["embtab"],
                    in_offset=bass.IndirectOffsetOnAxis(ap=idxc[:, m:m + 1],
                                                        axis=0),
                )
            # emb^T tiles [E-chunk 128, TB]
            embT = [ip.tile([128, TB], bf, name=f"embT{k}") for k in range(KT)]
            for m in range(NW):
                for k in range(KT):
                    tp = psp.tile([128, 128], bf, tag="ht", bufs=2)
                    nc.tensor.transpose(tp[:], embm[m][:, 128 * k:128 * (k + 1)],
                                        id128[:])
                    nc.vector.tensor_copy(embT[k][:, 128 * m:128 * (m + 1)], tp[:])

            # Wx = emb @ W_ih^T + gbias   -> wx[m] [128, GC] bf16
            for m in range(NW):
                for nch in range(4):
                    sl = slice(512 * nch, 512 * (nch + 1))
                    wps = psp.tile([128, 512], f32, tag="fc", bufs=2)
                    for k in range(KT):
                        nc.tensor.matmul(wps[:], lhsT=embT[k][:, 128 * m:128 * (m + 1)],
                                         rhs=wih[k][:, sl], start=(k == 0), stop=False)
                    nc.tensor.matmul(wps[:], lhsT=onesr[:], rhs=gbias[:, sl],
                                     start=False, stop=True)
                    nc.vector.tensor_copy(wx[m][:, sl], wps[:])

        # ---- fc weights (own pool: reuses space released by the init pool)
        fcp = ctx.enter_context(tc.tile_pool(name="fcp" + pfx, bufs=1))
        fcwa = fcp.tile([128, KT * V], bf)
        nc.sync.dma_start(fcwa[:].rearrange("p (k n) -> p k n", k=KT),
                          d["fcw"].rearrange("(k p) n -> p k n", k=KT))
        fcw = [fcwa[:, k * V:(k + 1) * V] for k in range(KT)]

        fout = ctx.enter_context(tc.tile_pool(name="fout" + pfx, bufs=2))

        dma_engs = [nc.gpsimd, nc.sync]
        fc_state = {"osb": None, "ndma": 0}

        def fc_chunks(w, nv_lo, nv_hi):
            """Emit fc chunks [nv_lo, nv_hi) of window w (128 rows)."""
            for nv in range(nv_lo, nv_hi):
                if fc_state["osb"] is None:
                    fc_state["osb"] = fout.tile([128, FCG * VCH], bf,
                                                tag="fo", name="osb")
                osb = fc_state["osb"]
                vsl = slice(VCH * nv, VCH * (nv + 1))
                fps = psp.tile([128, VCH], f32, tag="fc", bufs=2)
                for k in range(KT):
                    nc.tensor.matmul(fps[:],
                                     lhsT=hall[k][:, 128 * w:128 * (w + 1)],
                                     rhs=fcw[k][:, vsl],
                                     start=(k == 0), stop=False)
                nc.tensor.matmul(fps[:], lhsT=mrow[:, 128 * w:128 * (w + 1)],
                                 rhs=fcb[:, vsl], start=False, stop=True)
                gi = nv % FCG
                oslice = osb[:, VCH * gi:VCH * (gi + 1)]
                if nv % 2 == 0:
                    nc.scalar.copy(oslice, fps[:])
                else:
                    nc.vector.tensor_copy(oslice, fps[:])
                if gi == FCG - 1:
                    nv0 = nv - FCG + 1
                    dst = d["preds"][:, 8 * w:8 * (w + 1),
                                     VCH * nv0:VCH * (nv + 1)]
                    eng = dma_engs[fc_state["ndma"] % len(dma_engs)]
                    eng.dma_start(dst.rearrange("b t n -> t b n"), osb[:])
                    fc_state["ndma"] += 1
                    fc_state["osb"] = None

        # ================= recurrence =================
        # fc work for window w is spread over the 8 steps of window w+1
        # (2-3 chunks per step), so it fills PE gaps without ever getting
        # priority over the critical-path recurrence matmuls.
        hprev = [h0T[k][:] for k in range(KT)]
        for t in range(T):
            m, j = t // 8, t % 8
            # gates psum in two double-buffered halves so next step's
            # wx "selector" matmuls (which need no h) can run early
            g_lo = psp.tile([BS, GC // 2], f32, tag="gates", bufs=2,
                            name="g_lo")
            g_hi = psp.tile([BS, GC // 2], f32, tag="gates", bufs=2,
                            name="g_hi")
            chunks = [(g_lo, 0), (g_lo, 1), (g_hi, 2), (g_hi, 3)]

            def gsl(ci):
                gt, c = chunks[ci]
                return gt[:, 512 * (c % 2):512 * (c % 2 + 1)]

            for ci in range(4):
                nc.tensor.matmul(gsl(ci), lhsT=id128[:, 16 * j:16 * (j + 1)],
                                 rhs=wx[m][:, 512 * ci:512 * (ci + 1)],
                                 start=True, stop=False)
            for ci in range(4):
                for k in range(KT):
                    nc.tensor.matmul(gsl(ci), lhsT=hprev[k],
                                     rhs=whh[k][:, 512 * ci:512 * (ci + 1)],
                                     start=False, stop=(k == KT - 1))
            # chunk order: 0=i, 1=g, 2=f, 3=o (host permuted the weights)
            sig_i = wk.tile([BS, H], f32, tag="si")
            nc.scalar.activation(sig_i[:], gsl(0), AF.Sigmoid)
            tnh_g = wk.tile([BS, H], f32, tag="tg")
            nc.scalar.activation(tnh_g[:], gsl(1), AF.Tanh)
            sig_f = wk.tile([BS, H], f32, tag="sf")
            nc.scalar.activation(sig_f[:], gsl(2), AF.Sigmoid)
            sig_o = wk.tile([BS, H], f32, tag="so")
            nc.scalar.activation(sig_o[:], gsl(3), AF.Sigmoid)

            # tail in 128-wide quarters: h^T tile k (and with it the next
            # step's k-th matmuls) becomes available early
            p1 = wk.tile([BS, H], f32, tag="p1")
            p2 = wk.tile([BS, H], f32, tag="p2")
            c_new = c_st[(t + 1) % 2]
            tnh_c = wk.tile([BS, H], f32, tag="tc")
            h = wk.tile([BS, H], f32, tag="h")
            for k in range(KT):
                q = slice(128 * k, 128 * (k + 1))
                nc.vector.tensor_mul(p1[:, q], sig_i[:, q], tnh_g[:, q])
                nc.vector.tensor_mul(p2[:, q], sig_f[:, q],
                                     c_st[t % 2][:, q])
                nc.vector.tensor_add(c_new[:, q], p1[:, q], p2[:, q])
                nc.scalar.activation(tnh_c[:, q], c_new[:, q], AF.Tanh)
                nc.vector.scalar_tensor_tensor(
                    h[:, q], tnh_c[:, q], mcol[:, t:t + 1], sig_o[:, q],
                    op0=mybir.AluOpType.mult, op1=mybir.AluOpType.mult)
                htp = psp.tile([128, BS], f32, tag="ht", bufs=2, name="htp")
                nc.tensor.transpose(htp[:], h[:, q], id16f[:])
                nc.vector.tensor_copy(hall[k][:, 16 * t:16 * (t + 1)],
                                      htp[:])
            hprev = [hall[k][:, 16 * t:16 * (t + 1)] for k in range(KT)]

            if t >= 8:
                w, jj = t // 8 - 1, t % 8
                fc_chunks(w, (jj * NVC) // 8, ((jj + 1) * NVC) // 8)
        # last window drains after the final step
        fc_chunks(NW - 1, 0, NVC)


def _build(rep=1):
    key = ("nc", rep)
    if key in _CACHE:
        return _CACHE[key]
    import concourse.bass as bass
    import concourse.tile as tile
    from concourse import bacc, mybir

    dt = mybir.dt
    nc = bacc.Bacc("TRN2", target_bir_lowering=False, debug=False,
                   num_devices=NCORES)

    def din(name, shape, dty):
        return nc.dram_tensor(name, shape, dty, kind="ExternalInput").ap()

    d = {
        "embtab": din("embtab", [V, E], dt.bfloat16),
        "idx": din("idx", [128, NW], dt.int32),
        "wih": din("wih", [E, GC], dt.bfloat16),
        "whh": din("whh", [H, GC], dt.bfloat16),
        "initw": din("initw", [E, 2 * H], dt.bfloat16),
        "fcw": din("fcw", [H, V], dt.bfloat16),
        "feat": din("feat", [BS, LF * E], dt.bfloat16),
        "bfconst": din("bfconst", [128, CBF], dt.bfloat16),
        "f32const": din("f32const", [BS, CF32], dt.float32),
        "preds": nc.dram_tensor("preds", [BS, T, V], dt.bfloat16,
                                kind="ExternalOutput").ap(),
    }

    with tile.TileContext(nc) as tc:
        _emit(nc, tc, tile, bass, mybir, d, rep=rep)
    nc.compile()
    _CACHE[key] = nc
    return nc


def _shared_inputs(embedding, W_ih, W_hh, b_ih, b_hh, fc_w, fc_b,
                   init_h_w, init_h_b, init_c_w, init_c_b):
    sh = {}
    sh["embtab"] = np.ascontiguousarray(embedding, dtype=np.float32).astype(BF)
    sh["wih"] = np.ascontiguousarray(W_ih[_PERM].T).astype(BF)
    sh["whh"] = np.ascontiguousarray(W_hh[_PERM].T).astype(BF)
    iw = np.concatenate([init_h_w, init_c_w], axis=0)  # [2H, 512]
    sh["initw"] = np.ascontiguousarray(iw.T / np.float32(LF)).astype(BF)
    sh["fcw"] = np.ascontiguousarray(fc_w.T).astype(BF)
    # packed bf16 const blob (per-core mrow patched in _core_inputs)
    blob = np.zeros((128, CBF), dtype=BF)
    blob[:, 0:128] = np.eye(128, dtype=np.float32).astype(BF)
    blob[0:16, 128:144] = np.eye(16, dtype=np.float32).astype(BF)
    blob[0, 144:272] = np.ones(128, dtype=np.float32).astype(BF)
    blob[0, 784:784 + GC] = (b_ih + b_hh)[_PERM].astype(BF)
    blob[0, 2832:2832 + 2 * H] = np.concatenate(
        [init_h_b, init_c_b]).astype(BF)
    blob[0, 3856:3856 + V] = np.asarray(fc_b).astype(BF)
    sh["bfconst"] = blob
    return sh


def _core_inputs(sh, features, captions, lengths, ci):
    b0 = BS * ci
    feat = np.ascontiguousarray(features[b0:b0 + BS], dtype=np.float32)
    cap = np.asarray(captions[b0:b0 + BS]).astype(np.int64)
    lens = np.asarray(lengths[b0:b0 + BS]).astype(np.int64)
    m = {}
    m.update(sh)
    m["feat"] = feat.reshape(BS, LF * E).astype(BF)
    # (t,b)-ordered gather indices as columns: idx[:, mi] = rows 128*mi..+128
    m["idx"] = np.ascontiguousarray(
        cap.T.reshape(NW, 128).T).astype(np.int32)
    # mask[b, t] = t < len[b]
    mask = (np.arange(T)[None, :] < lens[:, None]).astype(np.float32)  # [BS,T]
    blob = np.array(m["bfconst"])
    blob[0, 272:272 + TB] = mask.T.reshape(TB).astype(BF)
    m["bfconst"] = blob
    f32c = np.zeros((BS, CF32), dtype=np.float32)
    f32c[:, 0:T] = mask
    f32c[:, T:T + 16] = np.eye(16, dtype=np.float32)
    m["f32const"] = f32c
    return m


def _in_maps(inputs):
    sh = _shared_inputs(
        inputs["embedding"], inputs["W_ih"], inputs["W_hh"], inputs["b_ih"],
        inputs["b_hh"], inputs["fc_w"], inputs["fc_b"], inputs["init_h_w"],
        inputs["init_h_b"], inputs["init_c_w"], inputs["init_c_b"])
    return [
        _core_inputs(sh, inputs["features"], inputs["captions"],
                     inputs["lengths"], ci)
        for ci in range(NCORES)
    ]


def _run(inputs, trace=False):
    from concourse.bass_utils import run_bass_kernel_spmd
    nc = _build()
    res = run_bass_kernel_spmd(nc, _in_maps(inputs), list(range(NCORES)),
                               trace=trace)
    preds = np.concatenate(
        [np.asarray(r["preds"], dtype=np.float32) for r in res.results], axis=0)
    return preds, res


def kernel(**inputs):
    """Run on HW. The first execution after a fresh NEFF compile has been
    observed to crash the exec unit sporadically (and poison the in-process
    jax runtime), so the device run happens in a subprocess with retries."""
    if os.environ.get("_LSTM_KERNEL_CHILD"):
        preds, _ = _run(inputs, trace=False)
        return preds
    import subprocess
    import tempfile
    import pickle
    with tempfile.TemporaryDirectory() as td:
        fin = os.path.join(td, "in.pkl")
        fout_p = os.path.join(td, "out.npy")
        with open(fin, "wb") as f:
            pickle.dump({k: np.asarray(v) for k, v in inputs.items()}, f)
        code = (
            "import pickle,numpy as np,sys;"
            f"sys.path.insert(0,{os.path.dirname(os.path.abspath(__file__))!r});"
            "import kernel;"
            f"ins=pickle.load(open({fin!r},'rb'));"
            f"np.save({fout_p!r}, kernel.kernel(**ins))"
        )
        env = {**os.environ, "_LSTM_KERNEL_CHILD": "1"}
        last = None
        for attempt in range(3):
            r = subprocess.run([sys.executable, "-c", code], env=env,
                               capture_output=True, text=True)
            if r.returncode == 0 and os.path.exists(fout_p):
                return np.load(fout_p)
            last = r
        raise RuntimeError(
            f"kernel subprocess failed after retries:\n{last.stdout[-2000:]}"
            f"\n{last.stderr[-4000:]}")


def _timed_runner(nc, in_maps):
    """Build the same shard_map executable run_bass_via_pjrt uses, but keep it
    for repeated timed execution with device-resident inputs."""
    import jax
    import numpy as jnp_np
    from jax.sharding import Mesh, PartitionSpec, NamedSharding
    from jax.experimental.shard_map import shard_map
    from concourse import bass2jax, mybir
    from concourse.bass2jax import _bass_exec_p, partition_id_tensor

    bass2jax.install_neuronx_cc_hook()
    n_cores = len(in_maps)
    partition_name = (nc.partition_id_tensor.name
                      if nc.partition_id_tensor else None)
    in_names, out_names, out_avals, zero_outs = [], [], [], []
    for alloc in nc.m.functions[0].allocations:
        if not isinstance(alloc, mybir.MemoryLocationSet):
            continue
        name = alloc.memorylocations[0].name
        if alloc.kind == "ExternalInput":
            if name != partition_name:
                in_names.append(name)
        elif alloc.kind == "ExternalOutput":
            shape = tuple(alloc.tensor_shape)
            dtype = mybir.dt.np(alloc.dtype)
            out_names.append(name)
            out_avals.append(jax.core.ShapedArray(shape, dtype))
            zero_outs.append(np.zeros(shape, dtype))
    n_params = len(in_names)
    n_outs = len(out_avals)
    param_names = list(in_names)
    in_names = in_names + out_names
    if partition_name is not None:
        in_names.append(partition_name)

    def _body(*args):
        operands = list(args)
        if partition_name is not None:
            operands.append(partition_id_tensor())
        outs = _bass_exec_p.bind(
            *operands, out_avals=tuple(out_avals), in_names=tuple(in_names),
            out_names=tuple(out_names), lowering_input_output_aliases=(),
            sim_require_finite=True, sim_require_nnan=True, nc=nc)
        return tuple(outs)

    devices = jax.devices()[:n_cores]
    mesh = Mesh(np.asarray(devices), ("core",))
    spec = PartitionSpec("core")
    sharded = jax.jit(
        shard_map(_body, mesh=mesh, in_specs=(spec,) * (n_params + n_outs),
                  out_specs=(spec,) * n_outs, check_rep=False),
        donate_argnums=tuple(range(n_params, n_params + n_outs)),
        keep_unused=True)
    sh = NamedSharding(mesh, spec)
    concat_in = [
        jax.device_put(np.concatenate(
            [np.asarray(m[nm]) for m in in_maps], axis=0), sh)
        for nm in param_names
    ]
    zglobal = [np.zeros((n_cores * z.shape[0], *z.shape[1:]), z.dtype)
               for z in zero_outs]

    def run_once():
        zs = [jax.device_put(z, sh) for z in zglobal]
        import time as _t
        jax.block_until_ready(zs)
        t0 = _t.perf_counter()
        out = sharded(*concat_in, *zs)
        jax.block_until_ready(out)
        dt = _t.perf_counter() - t0
        return out, dt

    def unpack(out):
        return [
            {nm: np.asarray(out[i]).reshape(n_cores, *out_avals[i].shape)[c]
             for i, nm in enumerate(out_names)}
            for c in range(n_cores)
        ]

    return run_once, unpack


def bench(inputs, iters=6, rep=9):
    """HW timing via on-device amplification: the same program emitted once
    vs `rep` times back-to-back; (T_rep - T_1)/(rep-1) cancels the axon
    tunnel overhead (~80ms) and host-side constants.  Interleaved sampling
    shares the noise environment between the two variants."""
    maps = _in_maps(inputs)
    nc1 = _build(1)
    run1, unpack1 = _timed_runner(nc1, maps)
    ncR = _build(rep)
    runR, _ = _timed_runner(ncR, maps)
    t1s, tRs = [], []
    out = None
    run1(); runR()  # warmup
    for _ in range(max(iters, 20)):
        out, dt1 = run1()
        _, dtR = runR()
        t1s.append(dt1)
        tRs.append(dtR)
    preds = np.concatenate(
        [np.asarray(r["preds"], dtype=np.float32) for r in unpack1(out)],
        axis=0)
    est = (min(tRs) - min(t1s)) / (rep - 1) * 1e9
    print(f"[bench] rep1 walls (ms): {[round(t*1e3,2) for t in t1s]}")
    print(f"[bench] rep{rep} walls (ms): {[round(t*1e3,2) for t in tRs]}")
    return preds, int(est)


def _calibration_times(iters):
    """Trivial kernel through the identical path to estimate fixed overhead."""
    import concourse.bass as bass
    import concourse.tile as tile
    from concourse import bacc, mybir

    if "cal" not in _CACHE:
        dt = mybir.dt
        nc = bacc.Bacc("TRN2", target_bir_lowering=False, debug=False,
                       num_devices=NCORES)
        x = nc.dram_tensor("x", [128, 128], dt.float32,
                           kind="ExternalInput").ap()
        y = nc.dram_tensor("y", [128, 128], dt.float32,
                           kind="ExternalOutput").ap()
        with tile.TileContext(nc) as tc:
            with tc.tile_pool(name="p", bufs=1) as p:
                t = p.tile([128, 128], dt.float32)
                nc.sync.dma_start(t[:], x)
                nc.sync.dma_start(y, t[:])
        nc.compile()
        _CACHE["cal"] = nc
    ncc = _CACHE["cal"]
    maps = [{"x": np.zeros((128, 128), np.float32)} for _ in range(NCORES)]
    run_once, _ = _timed_runner(ncc, maps)
    return [run_once()[1] for _ in range(iters)]
